# revision 31
# baseline (speedup 1.0000x reference)
"""Trainium2 Bass kernel for nn_AirResistance.

out[b, t] = x[b, 0] * r**t,  r = 1 + (0.99 - 1.0) * delta_t,  out: (B, steps, 1) f32

Rank-1 structure: out = x ⊗ rpow. The power vector rpow is precomputed on the
host (tiny) and broadcast to all 128 SBUF partitions; output values are
produced with per-partition-scalar multiplies on the vector engine and
streamed to HBM. Batch dim B is sharded across the 8 NeuronCores (pure data
parallelism, no communication).

Raw Bass (manual semaphores): this toolchain's walrus enforces at most one
sync-wait command per instruction, so waits are standalone wait_ge
instructions and every producer increments exactly one semaphore. Slot reuse
is gated by per-slot semaphores (a single shared completion counter would
race: DMA completions interleave per-engine across transfers).

DMA layout: HWDGE fans a c-descriptor DMA over (largest divisor of c <= 16)
SDMA engines in equal consecutive groups, so only c=128 (one descriptor per
partition, all 16 engines) streams at line rate. Steady-state groups cover
384 output rows with partition p holding rows 3p..3p+2 (contiguous 48KB in
DRAM and SBUF) — large descriptors at full fan-out. Groups rotate over K=3
SBUF slots so a group only waits on DMAs from three groups back, and group
stores alternate between the SP and ACT HWDGE rings; some physical cores
have a ~25% slower SDMA engine 15, and the deep rotation keeps that
straggler from stalling compute.

Ramp: the rp table loads as two column-half DMAs and the first groups are
small (128/256/256 rows, group 0 stored as two column-half DMAs), so the
first output DMA issues right after the first rp half lands instead of after
a full-table load plus a full-size group compute.
"""

import numpy as np

import concourse.bass as bass
from concourse import mybir
from concourse.bass_utils import run_bass_kernel_spmd

N_CORES = 8
B = 32768
STEPS = 4096
N_RP_CHUNKS = 2                       # rp table loads in column halves
RP_CHUNK = STEPS // N_RP_CHUNKS
P = 128
ROWS_PER_CORE = B // N_CORES          # 4096
K = 3                                 # SBUF slots (48KB/partition each)
MAX_RPP = 3

# groups: (rpp, col_split) — rows = 128*rpp; col_split only for group 0.
# K=3 rotation means a group only waits on the DMAs from three groups back,
# so a straggling DMA engine never stalls the compute pipeline.
_GROUPS = [(1, True), (2, False), (2, False)] + [(3, False)] * 9
assert sum(r for r, _ in _GROUPS) * P == ROWS_PER_CORE

_nc_cache = None


def _group_meta():
    """Per group: row0, rpp, xt_col0, list of (j-range, col-range) sub-DMAs."""
    metas = []
    row0 = 0
    col0 = 0
    for rpp, col_split in _GROUPS:
        if col_split:
            subs = [
                (0, rpp, q * RP_CHUNK, (q + 1) * RP_CHUNK)
                for q in range(N_RP_CHUNKS)
            ]
        else:
            subs = [(0, rpp, 0, STEPS)]
        metas.append({"row0": row0, "rpp": rpp, "xt_col0": col0, "subs": subs})
        row0 += P * rpp
        col0 += rpp
    return metas


def _build_bass():
    f32 = mybir.dt.float32
    nc = bass.Bass("TRN2", target_bir_lowering=False, debug=False)

    metas = _group_meta()
    n_xt_cols = sum(m["rpp"] for m in metas)

    xt_d = nc.dram_tensor("xt", [P, n_xt_cols], f32, kind="ExternalInput").ap()
    rp_d = nc.dram_tensor("rp", [P, STEPS], f32, kind="ExternalInput").ap()
    out_d = nc.dram_tensor(
        "out", [ROWS_PER_CORE, STEPS], f32, kind="ExternalOutput"
    ).ap()

    rp_sb = nc.alloc_sbuf_tensor("rp_sb", [P, STEPS], f32).ap()
    xt_sb = nc.alloc_sbuf_tensor("xt_sb", [P, n_xt_cols], f32).ap()
    ot_sb = nc.alloc_sbuf_tensor("ot_sb", [P, K, MAX_RPP, STEPS], f32).ap()

    def group_ot(g):
        return ot_sb[:, g % K, :, :]

    # out AP for group g: partition p, row row0 + rpp*p + j, cols [c0:c1]
    def out_ap(m, j0, j1, c0, c1):
        rpp = m["rpp"]
        g_rows = out_d[m["row0"] : m["row0"] + P * rpp, :]
        # (p, j, t) with row = rpp*p + j
        g3 = g_rows.rearrange("(p j) t -> p j t", j=rpp)
        return g3[:, j0:j1, c0:c1]

    # TS op counts per group (for sem_cmp thresholds)
    ts_per_group = []
    for m in metas:
        n = 0
        for j0, j1, c0, c1 in m["subs"]:
            n += j1 - j0
        ts_per_group.append(n)
    cum_ts = np.concatenate([[0], np.cumsum(ts_per_group)])

    # group g -> slot sem value once its DMAs complete
    slot_after_group = {}
    run = {s: 0 for s in range(K)}
    for g, m in enumerate(metas):
        run[g % K] += 16 * len(m["subs"])
        slot_after_group[g] = run[g % K]

    with (
        nc.Block() as block,
        nc.semaphore("sem_xt") as sem_xt,
        nc.semaphore("sem_r0") as sem_r0,
        nc.semaphore("sem_r1") as sem_r1,
        nc.semaphore("sem_cmp") as sem_cmp,
        nc.semaphore("sem_s0") as sem_s0,
        nc.semaphore("sem_s1") as sem_s1,
        nc.semaphore("sem_s2") as sem_s2,
    ):
        slot_sems = [sem_s0, sem_s1, sem_s2]
        rp_sems = [sem_r0, sem_r1]

        # group -> issuing queue: even groups on the SP HWDGE ring, odd on the
        # ACT HWDGE ring (two independent descriptor rings feed the SDMA
        # engines; splits per-ring FIFO pressure and hedges against per-core
        # slow engines behind one ring)
        def emit_group_dmas(eng, g, m, ts_before):
            done_ts = ts_before
            for j0, j1, c0, c1 in m["subs"]:
                done_ts += j1 - j0
                eng.wait_ge(sem_cmp, done_ts)
                eng.dma_start(
                    out=out_ap(m, j0, j1, c0, c1),
                    in_=group_ot(g)[:, j0:j1, c0:c1],
                ).then_inc(slot_sems[g % K], 16)

        @block.sync
        def _(sync):
            sync.dma_start(out=xt_sb, in_=xt_d).then_inc(sem_xt, 16)
            for q in range(N_RP_CHUNKS):
                cols = slice(q * RP_CHUNK, (q + 1) * RP_CHUNK)
                sync.dma_start(out=rp_sb[:, cols], in_=rp_d[:, cols]).then_inc(
                    rp_sems[q], 16
                )
            for g, m in enumerate(metas):
                if g % 2 == 0:
                    emit_group_dmas(sync, g, m, int(cum_ts[g]))
            for s in range(K):
                last_g = max(g for g in range(len(metas)) if g % K == s)
                sync.wait_ge(slot_sems[s], slot_after_group[last_g])

        @block.scalar
        def _(scalar):
            for g, m in enumerate(metas):
                if g % 2 == 1:
                    emit_group_dmas(scalar, g, m, int(cum_ts[g]))

        @block.vector
        def _(vector):
            vector.wait_ge(sem_xt, 16)
            quarters_waited = 0
            for g, m in enumerate(metas):
                if g >= K:
                    # slot g%K was last drained by the DMAs of group g-K
                    vector.wait_ge(slot_sems[g % K], slot_after_group[g - K])
                for j0, j1, c0, c1 in m["subs"]:
                    while quarters_waited * RP_CHUNK < c1:
                        vector.wait_ge(rp_sems[quarters_waited], 16)
                        quarters_waited += 1
                    for j in range(j0, j1):
                        vector.tensor_scalar_mul(
                            group_ot(g)[:, j, c0:c1],
                            rp_sb[:, c0:c1],
                            xt_sb[:, m["xt_col0"] + j : m["xt_col0"] + j + 1],
                        ).then_inc(sem_cmp, 1)

    return nc


def _get_nc():
    global _nc_cache
    if _nc_cache is None:
        _nc_cache = _build_bass()
    return _nc_cache


def make_in_maps(x, delta_t):
    x = np.asarray(x, dtype=np.float32)
    r32 = np.float32(1.0 + (0.99 - 1.0) * float(delta_t))
    rpow = (np.float64(r32) ** np.arange(STEPS, dtype=np.float64)).astype(np.float32)
    rp_b = np.ascontiguousarray(np.broadcast_to(rpow, (P, STEPS)))

    metas = _group_meta()
    n_xt_cols = sum(m["rpp"] for m in metas)

    in_maps = []
    for c in range(N_CORES):
        xs = x[c * ROWS_PER_CORE : (c + 1) * ROWS_PER_CORE, 0]
        # xt[p, col0+j] = x_shard[row0 + rpp*p + j]
        xt = np.zeros((P, n_xt_cols), dtype=np.float32)
        for m in metas:
            rpp = m["rpp"]
            blk = xs[m["row0"] : m["row0"] + P * rpp].reshape(P, rpp)
            xt[:, m["xt_col0"] : m["xt_col0"] + rpp] = blk
        in_maps.append({"xt": xt, "rp": rp_b})
    return in_maps


def kernel(steps, x, delta_t):
    steps = int(steps)
    x = np.asarray(x, dtype=np.float32)
    assert steps == STEPS and x.shape == (B, 1), (steps, x.shape)

    res = run_bass_kernel_spmd(
        _get_nc(), make_in_maps(x, delta_t), list(range(N_CORES))
    )
    out = np.concatenate([res.results[c]["out"] for c in range(N_CORES)], axis=0)
    return out.reshape(B, STEPS, 1)


# revision 34
# speedup vs baseline: 1.0009x; 1.0009x over previous
"""Trainium2 Bass kernel for nn_AirResistance.

out[b, t] = x[b, 0] * r**t,  r = 1 + (0.99 - 1.0) * delta_t,  out: (B, steps, 1) f32

Rank-1 structure: out = x ⊗ rpow. The power vector rpow is precomputed on the
host (tiny) and broadcast to all 128 SBUF partitions; output values are
produced with per-partition-scalar multiplies on the vector engine and
streamed to HBM. Batch dim B is sharded across the 8 NeuronCores (pure data
parallelism, no communication).

Raw Bass (manual semaphores): this toolchain's walrus enforces at most one
sync-wait command per instruction, so waits are standalone wait_ge
instructions and every producer increments exactly one semaphore. Slot reuse
is gated by per-slot semaphores (a single shared completion counter would
race: DMA completions interleave per-engine across transfers).

DMA layout: HWDGE fans a c-descriptor DMA over (largest divisor of c <= 16)
SDMA engines in equal consecutive groups, so only c=128 (one descriptor per
partition, all 16 engines) streams at line rate. Steady-state groups cover
384 output rows with partition p holding rows 3p..3p+2 (contiguous 48KB in
DRAM and SBUF) — large descriptors at full fan-out. Groups rotate over K=3
SBUF slots so a group only waits on DMAs from three groups back, and group
stores alternate between the SP and ACT HWDGE rings; some physical cores
have a ~25% slower SDMA engine 15, and the deep rotation keeps that
straggler from stalling compute.

Ramp: the rp table loads as two column-half DMAs and the first groups are
small (128/256/256 rows, group 0 stored as two column-half DMAs), so the
first output DMA issues right after the first rp half lands instead of after
a full-table load plus a full-size group compute.
"""

import numpy as np

import concourse.bass as bass
from concourse import mybir
from concourse.bass_utils import run_bass_kernel_spmd

N_CORES = 8
B = 32768
STEPS = 4096
N_RP_CHUNKS = 2                       # rp table loads in column halves
RP_CHUNK = STEPS // N_RP_CHUNKS
P = 128
ROWS_PER_CORE = B // N_CORES          # 4096
K = 3                                 # SBUF slots (48KB/partition each)
MAX_RPP = 3

# groups: (rpp, col_split) — rows = 128*rpp; col_split only for group 0.
# K=3 rotation means a group only waits on the DMAs from three groups back,
# so a straggling DMA engine never stalls the compute pipeline.
_GROUPS = [(1, True), (2, False), (2, False)] + [(3, False)] * 9
assert sum(r for r, _ in _GROUPS) * P == ROWS_PER_CORE

_nc_cache = None


def _group_meta():
    """Per group: row0, rpp, xt_col0, list of (j-range, col-range) sub-DMAs."""
    metas = []
    row0 = 0
    col0 = 0
    for rpp, col_split in _GROUPS:
        if col_split:
            subs = [
                (0, rpp, q * RP_CHUNK, (q + 1) * RP_CHUNK)
                for q in range(N_RP_CHUNKS)
            ]
        else:
            subs = [(0, rpp, 0, STEPS)]
        metas.append({"row0": row0, "rpp": rpp, "xt_col0": col0, "subs": subs})
        row0 += P * rpp
        col0 += rpp
    return metas


def _build_bass():
    f32 = mybir.dt.float32
    nc = bass.Bass("TRN2", target_bir_lowering=False, debug=False)

    metas = _group_meta()
    n_xt_cols = sum(m["rpp"] for m in metas)

    xt_d = nc.dram_tensor("xt", [P, n_xt_cols], f32, kind="ExternalInput").ap()
    rp_d = nc.dram_tensor("rp", [P, STEPS], f32, kind="ExternalInput").ap()
    out_d = nc.dram_tensor(
        "out", [ROWS_PER_CORE, STEPS], f32, kind="ExternalOutput"
    ).ap()

    rp_sb = nc.alloc_sbuf_tensor("rp_sb", [P, STEPS], f32).ap()
    xt_sb = nc.alloc_sbuf_tensor("xt_sb", [P, n_xt_cols], f32).ap()
    ot_sb = nc.alloc_sbuf_tensor("ot_sb", [P, K, MAX_RPP, STEPS], f32).ap()

    def group_ot(g):
        return ot_sb[:, g % K, :, :]

    # out AP for group g: partition p, row row0 + rpp*p + j, cols [c0:c1]
    def out_ap(m, j0, j1, c0, c1):
        rpp = m["rpp"]
        g_rows = out_d[m["row0"] : m["row0"] + P * rpp, :]
        # (p, j, t) with row = rpp*p + j
        g3 = g_rows.rearrange("(p j) t -> p j t", j=rpp)
        return g3[:, j0:j1, c0:c1]

    # TS op counts per group (for sem_cmp thresholds)
    ts_per_group = []
    for m in metas:
        n = 0
        for j0, j1, c0, c1 in m["subs"]:
            n += j1 - j0
        ts_per_group.append(n)
    cum_ts = np.concatenate([[0], np.cumsum(ts_per_group)])

    # group g -> slot sem value once its DMAs complete
    slot_after_group = {}
    run = {s: 0 for s in range(K)}
    for g, m in enumerate(metas):
        run[g % K] += 16 * len(m["subs"])
        slot_after_group[g] = run[g % K]

    with (
        nc.Block() as block,
        nc.semaphore("sem_xt") as sem_xt,
        nc.semaphore("sem_r0") as sem_r0,
        nc.semaphore("sem_r1") as sem_r1,
        nc.semaphore("sem_cmp") as sem_cmp,
        nc.semaphore("sem_s0") as sem_s0,
        nc.semaphore("sem_s1") as sem_s1,
        nc.semaphore("sem_s2") as sem_s2,
    ):
        slot_sems = [sem_s0, sem_s1, sem_s2]
        rp_sems = [sem_r0, sem_r1]

        # group -> issuing queue: even groups on the SP HWDGE ring, odd on the
        # ACT HWDGE ring (two independent descriptor rings feed the SDMA
        # engines; splits per-ring FIFO pressure and hedges against per-core
        # slow engines behind one ring)
        def emit_group_dmas(eng, g, m, ts_before):
            done_ts = ts_before
            for j0, j1, c0, c1 in m["subs"]:
                done_ts += j1 - j0
                eng.wait_ge(sem_cmp, done_ts)
                eng.dma_start(
                    out=out_ap(m, j0, j1, c0, c1),
                    in_=group_ot(g)[:, j0:j1, c0:c1],
                ).then_inc(slot_sems[g % K], 16)

        @block.sync
        def _(sync):
            sync.dma_start(out=xt_sb, in_=xt_d).then_inc(sem_xt, 16)
            lo = slice(0, RP_CHUNK)
            sync.dma_start(out=rp_sb[:, lo], in_=rp_d[:, lo]).then_inc(
                rp_sems[0], 16
            )
            for g, m in enumerate(metas):
                if g % 2 == 0:
                    emit_group_dmas(sync, g, m, int(cum_ts[g]))
            for s in range(K):
                last_g = max(g for g in range(len(metas)) if g % K == s)
                sync.wait_ge(slot_sems[s], slot_after_group[last_g])

        @block.scalar
        def _(scalar):
            # upper rp half rides the ACT ring, concurrent with the lower half
            hi = slice(RP_CHUNK, STEPS)
            scalar.dma_start(out=rp_sb[:, hi], in_=rp_d[:, hi]).then_inc(
                rp_sems[1], 16
            )
            for g, m in enumerate(metas):
                if g % 2 == 1:
                    emit_group_dmas(scalar, g, m, int(cum_ts[g]))

        @block.vector
        def _(vector):
            vector.wait_ge(sem_xt, 16)
            quarters_waited = 0
            for g, m in enumerate(metas):
                if g >= K:
                    # slot g%K was last drained by the DMAs of group g-K
                    vector.wait_ge(slot_sems[g % K], slot_after_group[g - K])
                for j0, j1, c0, c1 in m["subs"]:
                    while quarters_waited * RP_CHUNK < c1:
                        vector.wait_ge(rp_sems[quarters_waited], 16)
                        quarters_waited += 1
                    for j in range(j0, j1):
                        vector.tensor_scalar_mul(
                            group_ot(g)[:, j, c0:c1],
                            rp_sb[:, c0:c1],
                            xt_sb[:, m["xt_col0"] + j : m["xt_col0"] + j + 1],
                        ).then_inc(sem_cmp, 1)

    return nc


def _get_nc():
    global _nc_cache
    if _nc_cache is None:
        _nc_cache = _build_bass()
    return _nc_cache


def make_in_maps(x, delta_t):
    x = np.asarray(x, dtype=np.float32)
    r32 = np.float32(1.0 + (0.99 - 1.0) * float(delta_t))
    rpow = (np.float64(r32) ** np.arange(STEPS, dtype=np.float64)).astype(np.float32)
    rp_b = np.ascontiguousarray(np.broadcast_to(rpow, (P, STEPS)))

    metas = _group_meta()
    n_xt_cols = sum(m["rpp"] for m in metas)

    in_maps = []
    for c in range(N_CORES):
        xs = x[c * ROWS_PER_CORE : (c + 1) * ROWS_PER_CORE, 0]
        # xt[p, col0+j] = x_shard[row0 + rpp*p + j]
        xt = np.zeros((P, n_xt_cols), dtype=np.float32)
        for m in metas:
            rpp = m["rpp"]
            blk = xs[m["row0"] : m["row0"] + P * rpp].reshape(P, rpp)
            xt[:, m["xt_col0"] : m["xt_col0"] + rpp] = blk
        in_maps.append({"xt": xt, "rp": rp_b})
    return in_maps


def kernel(steps, x, delta_t):
    steps = int(steps)
    x = np.asarray(x, dtype=np.float32)
    assert steps == STEPS and x.shape == (B, 1), (steps, x.shape)

    res = run_bass_kernel_spmd(
        _get_nc(), make_in_maps(x, delta_t), list(range(N_CORES))
    )
    out = np.concatenate([res.results[c]["out"] for c in range(N_CORES)], axis=0)
    return out.reshape(B, STEPS, 1)


# revision 37
# speedup vs baseline: 1.0464x; 1.0455x over previous
"""Trainium2 Bass kernel for nn_AirResistance.

out[b, t] = x[b, 0] * r**t,  r = 1 + (0.99 - 1.0) * delta_t,  out: (B, steps, 1) f32

Rank-1 structure: out = x ⊗ rpow. The power vector rpow is precomputed on the
host (tiny) and broadcast to all 128 SBUF partitions; output values are
produced with per-partition-scalar multiplies on the vector engine and
streamed to HBM. Batch dim B is sharded across the 8 NeuronCores (pure data
parallelism, no communication).

Raw Bass (manual semaphores): this toolchain's walrus enforces at most one
sync-wait command per instruction, so waits are standalone wait_ge
instructions and every producer increments exactly one semaphore. Slot reuse
is gated by per-slot semaphores (a single shared completion counter would
race: DMA completions interleave per-engine across transfers).

DMA layout: HWDGE fans a c-descriptor DMA over (largest divisor of c <= 16)
SDMA engines in equal consecutive groups, so only c=128 (one descriptor per
partition, all 16 engines) streams at line rate. Steady-state groups cover
384 output rows with partition p holding rows 3p..3p+2 (contiguous 48KB in
DRAM and SBUF) — large descriptors at full fan-out. Groups rotate over K=3
SBUF slots so a group only waits on DMAs from three groups back, and group
stores alternate between the SP and ACT HWDGE rings; some physical cores
have a ~25% slower SDMA engine 15, and the deep rotation keeps that
straggler from stalling compute.

Ramp: the rp table loads as two column-half DMAs and the first groups are
small (128/256/256 rows, group 0 stored as two column-half DMAs), so the
first output DMA issues right after the first rp half lands instead of after
a full-table load plus a full-size group compute.
"""

import numpy as np

import concourse.bass as bass
from concourse import mybir
from concourse.bass_utils import run_bass_kernel_spmd

N_CORES = 8
B = 32768
STEPS = 4096
N_RP_CHUNKS = 2                       # rp table loads in column halves
RP_CHUNK = STEPS // N_RP_CHUNKS
P = 128
ROWS_PER_CORE = B // N_CORES          # 4096
K = 3                                 # SBUF slots (48KB/partition each)
MAX_RPP = 3

# groups: (rpp, col_split) — rows = 128*rpp; col_split only for group 0.
# K=3 rotation means a group only waits on the DMAs from three groups back,
# so a straggling DMA engine never stalls the compute pipeline.
_GROUPS = [(1, True), (2, False), (2, False)] + [(3, False)] * 9
assert sum(r for r, _ in _GROUPS) * P == ROWS_PER_CORE

_nc_cache = None


def _group_meta():
    """Per group: row0, rpp, xt_col0, list of (j-range, col-range) sub-DMAs."""
    metas = []
    row0 = 0
    col0 = 0
    for rpp, col_split in _GROUPS:
        if col_split:
            subs = [
                (0, rpp, q * RP_CHUNK, (q + 1) * RP_CHUNK)
                for q in range(N_RP_CHUNKS)
            ]
        else:
            subs = [(0, rpp, 0, STEPS)]
        metas.append({"row0": row0, "rpp": rpp, "xt_col0": col0, "subs": subs})
        row0 += P * rpp
        col0 += rpp
    return metas


def _build_bass():
    f32 = mybir.dt.float32
    nc = bass.Bass(
        "TRN2", target_bir_lowering=False, debug=False, monotonic_sem_count=0
    )

    metas = _group_meta()
    n_xt_cols = sum(m["rpp"] for m in metas)

    xt_d = nc.dram_tensor("xt", [P, n_xt_cols], f32, kind="ExternalInput").ap()
    rp_d = nc.dram_tensor("rp", [P, STEPS], f32, kind="ExternalInput").ap()
    out_d = nc.dram_tensor(
        "out", [ROWS_PER_CORE, STEPS], f32, kind="ExternalOutput"
    ).ap()

    rp_sb = nc.alloc_sbuf_tensor("rp_sb", [P, STEPS], f32).ap()
    xt_sb = nc.alloc_sbuf_tensor("xt_sb", [P, n_xt_cols], f32).ap()
    ot_sb = nc.alloc_sbuf_tensor("ot_sb", [P, K, MAX_RPP, STEPS], f32).ap()

    def group_ot(g):
        return ot_sb[:, g % K, :, :]

    # out AP for group g: partition p, row row0 + rpp*p + j, cols [c0:c1]
    def out_ap(m, j0, j1, c0, c1):
        rpp = m["rpp"]
        g_rows = out_d[m["row0"] : m["row0"] + P * rpp, :]
        # (p, j, t) with row = rpp*p + j
        g3 = g_rows.rearrange("(p j) t -> p j t", j=rpp)
        return g3[:, j0:j1, c0:c1]

    # TS op counts per group (for sem_cmp thresholds)
    ts_per_group = []
    for m in metas:
        n = 0
        for j0, j1, c0, c1 in m["subs"]:
            n += j1 - j0
        ts_per_group.append(n)
    cum_ts = np.concatenate([[0], np.cumsum(ts_per_group)])

    # group g -> slot sem value once its DMAs complete
    slot_after_group = {}
    run = {s: 0 for s in range(K)}
    for g, m in enumerate(metas):
        run[g % K] += 16 * len(m["subs"])
        slot_after_group[g] = run[g % K]

    with (
        nc.Block() as block,
        nc.semaphore("sem_r0") as sem_r0,
        nc.semaphore("sem_r1") as sem_r1,
        nc.semaphore("sem_cmp") as sem_cmp,
        nc.semaphore("sem_s0") as sem_s0,
        nc.semaphore("sem_s1") as sem_s1,
        nc.semaphore("sem_s2") as sem_s2,
    ):
        slot_sems = [sem_s0, sem_s1, sem_s2]
        rp_sems = [sem_r0, sem_r1]

        # group -> issuing queue: even groups on the SP HWDGE ring, odd on the
        # ACT HWDGE ring (two independent descriptor rings feed the SDMA
        # engines; splits per-ring FIFO pressure and hedges against per-core
        # slow engines behind one ring)
        def emit_group_dmas(eng, g, m, ts_before):
            done_ts = ts_before
            for j0, j1, c0, c1 in m["subs"]:
                done_ts += j1 - j0
                eng.wait_ge(sem_cmp, done_ts)
                eng.dma_start(
                    out=out_ap(m, j0, j1, c0, c1),
                    in_=group_ot(g)[:, j0:j1, c0:c1],
                ).then_inc(slot_sems[g % K], 16)

        @block.sync
        def _(sync):
            # xt shares sem_r0 with the rp lower half (both on this ring;
            # threshold 32 means both transfers fully completed)
            sync.dma_start(out=xt_sb, in_=xt_d).then_inc(sem_r0, 16)
            lo = slice(0, RP_CHUNK)
            sync.dma_start(out=rp_sb[:, lo], in_=rp_d[:, lo]).then_inc(
                rp_sems[0], 16
            )
            for g, m in enumerate(metas):
                if g % 2 == 0:
                    emit_group_dmas(sync, g, m, int(cum_ts[g]))
            for s in range(K):
                last_g = max(g for g in range(len(metas)) if g % K == s)
                sync.wait_ge(slot_sems[s], slot_after_group[last_g])

        @block.scalar
        def _(scalar):
            # upper rp half rides the ACT ring, concurrent with the lower half
            hi = slice(RP_CHUNK, STEPS)
            scalar.dma_start(out=rp_sb[:, hi], in_=rp_d[:, hi]).then_inc(
                rp_sems[1], 16
            )
            for g, m in enumerate(metas):
                if g % 2 == 1:
                    emit_group_dmas(scalar, g, m, int(cum_ts[g]))

        @block.vector
        def _(vector):
            rp_thresholds = [32, 16]  # chunk 0's sem also counts the xt load
            quarters_waited = 0
            for g, m in enumerate(metas):
                if g >= K:
                    # slot g%K was last drained by the DMAs of group g-K
                    vector.wait_ge(slot_sems[g % K], slot_after_group[g - K])
                for j0, j1, c0, c1 in m["subs"]:
                    while quarters_waited * RP_CHUNK < c1:
                        vector.wait_ge(
                            rp_sems[quarters_waited],
                            rp_thresholds[quarters_waited],
                        )
                        quarters_waited += 1
                    for j in range(j0, j1):
                        vector.tensor_scalar_mul(
                            group_ot(g)[:, j, c0:c1],
                            rp_sb[:, c0:c1],
                            xt_sb[:, m["xt_col0"] + j : m["xt_col0"] + j + 1],
                        ).then_inc(sem_cmp, 1)

    return nc


def _get_nc():
    global _nc_cache
    if _nc_cache is None:
        _nc_cache = _build_bass()
    return _nc_cache


def make_in_maps(x, delta_t):
    x = np.asarray(x, dtype=np.float32)
    r32 = np.float32(1.0 + (0.99 - 1.0) * float(delta_t))
    rpow = (np.float64(r32) ** np.arange(STEPS, dtype=np.float64)).astype(np.float32)
    rp_b = np.ascontiguousarray(np.broadcast_to(rpow, (P, STEPS)))

    metas = _group_meta()
    n_xt_cols = sum(m["rpp"] for m in metas)

    in_maps = []
    for c in range(N_CORES):
        xs = x[c * ROWS_PER_CORE : (c + 1) * ROWS_PER_CORE, 0]
        # xt[p, col0+j] = x_shard[row0 + rpp*p + j]
        xt = np.zeros((P, n_xt_cols), dtype=np.float32)
        for m in metas:
            rpp = m["rpp"]
            blk = xs[m["row0"] : m["row0"] + P * rpp].reshape(P, rpp)
            xt[:, m["xt_col0"] : m["xt_col0"] + rpp] = blk
        in_maps.append({"xt": xt, "rp": rp_b})
    return in_maps


def kernel(steps, x, delta_t):
    steps = int(steps)
    x = np.asarray(x, dtype=np.float32)
    assert steps == STEPS and x.shape == (B, 1), (steps, x.shape)

    res = run_bass_kernel_spmd(
        _get_nc(), make_in_maps(x, delta_t), list(range(N_CORES))
    )
    out = np.concatenate([res.results[c]["out"] for c in range(N_CORES)], axis=0)
    return out.reshape(B, STEPS, 1)
